# revision 11
# baseline (speedup 1.0000x reference)
"""Attention with full pair dedup: K and V each computed for own half
only and exchanged via 2-rank AllGathers (v4).

Per-core matmul streaming floor is ~0.505 ns/col on this part; the v3
baseline ran at that floor for its instruction mix, so the only wins are
fewer streamed columns and shorter lead-in/tail. v4 cuts the redundant
partner-half K projection (36864 cols ~ 18.6us) by gathering K like V.

Schedule (times at 0.505 ns/col, lead-in ~8.5us):
  warm collective (absorbs the ~25us CC-stream setup barrier seen in the
    v3 trace; triggered at ~1us off the critical path)
  KT-own slab0 -> gather-Ka (trigger ~19us, lands ~40us)
  KT-own slab1 -> gather-Kb (trigger ~28us, lands ~55us)
  V' -> gather-V (trigger ~47us, lands ~90us)
  QT (ends ~64us) -> scores (need K' full: ~57us < 64us ok)
  ... -> PV (needs V': ~90us < 114us ok)

Slot discipline: gather output is rank-ordered == physical key order
(rank0 = first half) on both cores, so kt/vp/et chunk indexing is
rank-agnostic everywhere.

Output staged fp16 (halves the out DMA + DVE normalize cost; adds
~2e-4 rel err against a 2e-2 gate)."""

import numpy as np

import concourse.bass as bass
import concourse.mybir as mybir
import concourse.tile as tile
from concourse import bacc
from concourse.bass_utils import run_bass_kernel_spmd

N_CORES = 8
B, N, D, OUT = 4, 2048, 768, 768
NQ = N // 2
P = 128
DC = D // P
OC = OUT // P
KC = N // P
HKC = KC // 2  # k-chunks per half
F32 = mybir.dt.float32
FP16 = mybir.dt.float16
PAIRS = [[0, 1], [2, 3], [4, 5], [6, 7]]

Q_BLOCKS = [(0, 384), (384, 384), (768, 256)]
N_WARMUP = 14


def build_attention_nc():
    nc = bacc.Bacc("TRN2", target_bir_lowering=False, debug=False)
    xq = nc.dram_tensor("xq", [D, NQ], FP16, kind="ExternalInput")
    w = nc.dram_tensor("w", [3, D, OUT], FP16, kind="ExternalInput")
    out = nc.dram_tensor("out", [NQ, OUT], FP16, kind="ExternalOutput")

    with tile.TileContext(nc) as tc:
        with (
            tc.tile_pool(name="persist", bufs=1) as persist,
            tc.tile_pool(name="dpool", bufs=1, space="DRAM") as dpool,
        ):
            qt = persist.tile([P, OC, NQ], FP16)  # QT[o,q] (local only)
            kt = persist.tile([P, OC, N], FP16)  # KT[o,k] physical order
            vp = persist.tile([P, KC, OUT + 2], FP16)  # V' physical order

            kpb_in = dpool.tile([P, OC, NQ], FP16, name="kpb_in")
            kpb_out = dpool.tile([2, P, OC, NQ], FP16, name="kpb_out")
            vpb_in = dpool.tile([P, HKC, OUT + 2], FP16)
            vpb_out = dpool.tile([2, P, HKC, OUT + 2], FP16)

            ones_sc = persist.tile([P, 1], F32, name="ones_sc")
            nc.vector.memset(ones_sc, 1.0)
            zero_sc = persist.tile([P, 1], F32, name="zero_sc")
            nc.vector.memset(zero_sc, 0.0)

            with (
                tc.tile_pool(name="slabs", bufs=2) as slabs,
                tc.tile_pool(name="psa", bufs=7, space="PSUM") as psa,
                tc.tile_pool(name="wpool", bufs=1) as wpool,
                tc.tile_pool(name="stage", bufs=4) as stage,
            ):
                wk_sb = wpool.tile([P, DC, OUT], FP16)
                wq_sb = wpool.tile([P, DC, OUT], FP16)
                wv_sb = wpool.tile([P, DC, OUT], FP16)

                # HAM warmup while the first DMAs fly
                warm = wpool.tile([P, 512], FP16, name="warm")
                nc.gpsimd.memset(warm, 1.0)
                wps = psa.tile([P, 512], F32, name="wps", bufs=1)
                for i in range(N_WARMUP):
                    nc.tensor.matmul(
                        wps,
                        warm[:, 0:P],
                        warm,
                        start=(i == 0),
                        stop=(i == N_WARMUP - 1),
                    )

                # DMAs in strict need-order (early DMA bandwidth ramps
                # slowly, so first-needed data must have nothing queued
                # ahead): wk+slab0 interleaved on sync/gpsimd, then
                # slab1, wv (V'), wq (QT).
                qslab_tiles = []
                for s in range(2):
                    qslab = slabs.tile(
                        [P, DC, 512], FP16, tag="slab", name=f"qslab{s}"
                    )
                    qslab_tiles.append(qslab)
                for dc in range(DC):
                    ew = nc.sync if dc % 2 == 0 else nc.gpsimd
                    ex = nc.gpsimd if dc % 2 == 0 else nc.sync
                    ew.dma_start(
                        out=wk_sb[:, dc, :], in_=w[1][dc * P : (dc + 1) * P, :]
                    )
                    ex.dma_start(
                        out=qslab_tiles[0][:, dc, :],
                        in_=xq[dc * P : (dc + 1) * P, 0:512],
                    )
                for dc in range(DC):
                    eng = nc.sync if dc % 2 == 0 else nc.gpsimd
                    eng.dma_start(
                        out=qslab_tiles[1][:, dc, :],
                        in_=xq[dc * P : (dc + 1) * P, 512:1024],
                    )
                for dc in range(DC):
                    eng = nc.gpsimd if dc % 2 == 0 else nc.sync
                    eng.dma_start(
                        out=wv_sb[:, dc, :], in_=w[2][dc * P : (dc + 1) * P, :]
                    )
                for dc in range(DC):
                    eng = nc.sync if dc % 2 == 0 else nc.gpsimd
                    eng.dma_start(
                        out=wq_sb[:, dc, :], in_=w[0][dc * P : (dc + 1) * P, :]
                    )

                # ---- A1: KT own half -> single pair gather ----
                for s in range(2):
                    slab = qslab_tiles[s]
                    for oc in range(OC):
                        ps = psa.tile([P, 512], F32, tag="psa")
                        for dc in range(DC):
                            nc.tensor.matmul(
                                ps,
                                wk_sb[:, dc, oc * P : (oc + 1) * P],
                                slab[:, dc, :],
                                start=(dc == 0),
                                stop=(dc == DC - 1),
                            )
                        kst = stage.tile([P, 512], FP16, tag="kst", bufs=6)
                        nc.vector.tensor_copy(kst, ps)
                        eng = nc.gpsimd if oc % 2 == 0 else nc.sync
                        eng.dma_start(
                            out=kpb_in[:, oc, s * 512 : (s + 1) * 512], in_=kst
                        )
                nc.gpsimd.collective_compute(
                    "AllGather",
                    mybir.AluOpType.bypass,
                    replica_groups=PAIRS,
                    ins=[kpb_in.opt()],
                    outs=[kpb_out.opt()],
                )
                # readback split 3 ways per rank-block (each DMA queue
                # sustains only ~110GB/s); scores consume h=0 (keys 0-1023)
                # first, so both h=0 chunks land in parallel ahead of h=1.
                # scalar is idle until the exp activations start.
                for h in range(2):
                    for og in range(3):
                        eng = [nc.sync, nc.scalar, nc.gpsimd][og]
                        eng.dma_start(
                            out=kt[
                                :, og * 2 : (og + 1) * 2, h * NQ : (h + 1) * NQ
                            ],
                            in_=kpb_out[h][:, og * 2 : (og + 1) * 2, :],
                        )

                # ---- A2: V' half -> gather ----
                for s in range(2):
                    slab = qslab_tiles[s]
                    for j in range(4):
                        kc = s * 4 + j
                        ps1 = psa.tile([P, 512], F32, tag="psa")
                        ps2 = psa.tile([P, 512], F32, tag="psa")
                        for dc in range(DC):
                            nc.tensor.matmul(
                                ps1[:, 0:384],
                                slab[:, dc, j * P : (j + 1) * P],
                                wv_sb[:, dc, 0:384],
                                start=(dc == 0),
                                stop=(dc == DC - 1),
                            )
                            nc.tensor.matmul(
                                ps2[:, 0:384],
                                slab[:, dc, j * P : (j + 1) * P],
                                wv_sb[:, dc, 384:OUT],
                                start=(dc == 0),
                                stop=(dc == DC - 1),
                            )
                        vst = stage.tile([P, OUT + 2], FP16, tag="vst", bufs=9)
                        nc.vector.tensor_copy(vst[:, 0:384], ps1[:, 0:384])
                        nc.vector.tensor_copy(vst[:, 384:OUT], ps2[:, 0:384])
                        nc.vector.tensor_copy(vst[:, OUT : OUT + 1], ones_sc)
                        nc.vector.tensor_copy(
                            vst[:, OUT + 1 : OUT + 2], zero_sc
                        )
                        nc.gpsimd.dma_start(out=vpb_in[:, kc, :], in_=vst)
                nc.gpsimd.collective_compute(
                    "AllGather",
                    mybir.AluOpType.bypass,
                    replica_groups=PAIRS,
                    ins=[vpb_in.opt()],
                    outs=[vpb_out.opt()],
                )
                for h in range(2):
                    eng = nc.sync if h == 0 else nc.gpsimd
                    eng.dma_start(
                        out=vp[:, h * HKC : (h + 1) * HKC, :], in_=vpb_out[h]
                    )

                # ---- A3: QT half (local) ----
                for s in range(2):
                    slab = qslab_tiles[s]
                    for oc in range(OC):
                        ps = psa.tile([P, 512], F32, tag="psa")
                        for dc in range(DC):
                            nc.tensor.matmul(
                                ps,
                                wq_sb[:, dc, oc * P : (oc + 1) * P],
                                slab[:, dc, :],
                                start=(dc == 0),
                                stop=(dc == DC - 1),
                            )
                        nc.vector.tensor_copy(
                            qt[:, oc, s * 512 : (s + 1) * 512], ps
                        )

            # ---- phase B: all scoresT runs, then all out runs ----
            with (
                tc.tile_pool(name="expp", bufs=50) as expp,
                tc.tile_pool(name="obp", bufs=3) as obp,
                tc.tile_pool(name="smallp", bufs=4) as smallp,
                tc.tile_pool(name="ps_sc", bufs=2, space="PSUM") as ps_sc,
                tc.tile_pool(name="ps_out", bufs=3, space="PSUM") as ps_out,
            ):
                ets = {}
                for bi, (q0, qb) in enumerate(Q_BLOCKS):
                    for kc in range(KC):
                        st = ps_sc.tile([P, 384], F32, tag="sc")
                        for oc in range(OC):
                            nc.tensor.matmul(
                                st[:, 0:qb],
                                kt[:, oc, kc * P : (kc + 1) * P],
                                qt[:, oc, q0 : q0 + qb],
                                start=(oc == 0),
                                stop=(oc == OC - 1),
                            )
                        et = expp.tile(
                            [P, 384], FP16, tag="exp", name=f"et{bi}_{kc}"
                        )
                        nc.scalar.activation(
                            et[:, 0:qb],
                            st[:, 0:qb],
                            mybir.ActivationFunctionType.Exp,
                            scale=0.125,
                        )
                        ets[(bi, kc)] = et
                for bi, (q0, qb) in enumerate(Q_BLOCKS):
                    nqc = qb // P
                    for j in range(nqc):
                        op = ps_out.tile(
                            [P, OUT + 2], F32, tag="out", name=f"outps{bi}_{j}"
                        )
                        for kc in range(KC):
                            nc.tensor.matmul(
                                op[:, 0:512],
                                ets[(bi, kc)][:, j * P : (j + 1) * P],
                                vp[:, kc, 0:512],
                                start=(kc == 0),
                                stop=(kc == KC - 1),
                            )
                        for kc in range(KC):
                            nc.tensor.matmul(
                                op[:, 512 : OUT + 2],
                                ets[(bi, kc)][:, j * P : (j + 1) * P],
                                vp[:, kc, 512 : OUT + 2],
                                start=(kc == 0),
                                stop=(kc == KC - 1),
                            )
                        recip = smallp.tile([P, 1], F32, tag="recip")
                        nc.vector.reciprocal(recip, op[:, OUT : OUT + 1])
                        ob = obp.tile([P, OUT], FP16, tag="ob")
                        nc.vector.tensor_scalar_mul(ob, op[:, 0:OUT], recip)
                        nc.sync.dma_start(
                            out=out[q0 + j * P : q0 + (j + 1) * P, :], in_=ob
                        )
    nc.finalize()
    return nc


_NC_CACHE = None


def _get_nc():
    global _NC_CACHE
    if _NC_CACHE is None:
        _NC_CACHE = build_attention_nc()
    return _NC_CACHE


def make_in_maps(x, kernel):
    x = np.asarray(x, dtype=np.float32)
    w = np.ascontiguousarray(
        np.asarray(kernel, dtype=np.float32).astype(np.float16)
    )
    in_maps = []
    for core in range(N_CORES):
        b, half = core // 2, core % 2
        xt16 = x[b].T.astype(np.float16)
        xq = np.ascontiguousarray(xt16[:, half * NQ : (half + 1) * NQ])
        in_maps.append({"xq": xq, "w": w})
    return in_maps


def assemble_output(results):
    out = np.empty((B, N, OUT), dtype=np.float32)
    for core in range(N_CORES):
        b, half = core // 2, core % 2
        out[b, half * NQ : (half + 1) * NQ, :] = results[core]["out"].astype(
            np.float32
        )
    return out


def run_on_hw(x, kernel, trace=False):
    nc = _get_nc()
    res = run_bass_kernel_spmd(
        nc, make_in_maps(x, kernel), list(range(N_CORES)), trace=trace
    )
    return assemble_output(res.results), res


def kernel(x, kernel):
    out, _ = run_on_hw(x, kernel, trace=False)
    return out
